# revision 26
# baseline (speedup 1.0000x reference)
"""Multi-head causal attention (GPT-2 style) on 8 TRN2 NeuronCores.

Sharding: core i handles batch i//2 and head-group i%2 (8 of 16 heads,
i.e. a 512-wide slice of the QKV projections and of the Wp rows).  Each
core computes a partial output-projection for its batch; partials from
the two cores of a batch are summed on the host (cheap 4MB adds), along
with the exactly-factored bias terms:
  - bq is added to Q on-device (affects scores per key-column),
  - bk is dropped (adds a per-query constant to scores: softmax-invariant),
  - bv and bp commute through attention (rows of attn sum to 1):
    y += bv @ Wp + bp, applied on host.

On-chip layout (per core), T=1024, C=1024, DH=64:
  xT   [C, T]   x transposed (host-side transpose)         -> rhs / lhsT
  Q^T  [512, T] = (Wq_s*s)^T x^T  (s=1/8 folded into Wq)   -> scores rhs
  K^T  [512, T]                                            -> scores lhsT
  V    [T, 8, 65] natural layout + ones column             -> ctx lhsT
  S^T  [k-tile 128, q-chunk 512] scores transposed: softmax denominator
       comes out of the ctx matmul via the ones column of V; causal mask
       applied as an elementwise multiply on exp(S^T) diagonal blocks.
  ctx^T[512, T] normalized context                         -> yproj lhsT
All matmuls run in float32r (1 cycle/row on the PE at N=512; ~1e-4
relative accuracy), accumulation in fp32 PSUM.
"""
import numpy as np

import concourse.bacc as bacc
import concourse.mybir as mybir
import concourse.tile as tile
from concourse.bass_utils import run_bass_kernel_spmd

B, T, C, H, DH = 4, 1024, 1024, 16, 64
P = 128
CS = 512            # per-core head-slice width (8 heads * 64)
F32 = mybir.dt.float32
F16 = mybir.dt.float16
F32R = mybir.dt.float32r
BF16 = mybir.dt.bfloat16
MM_DTYPE = BF16     # matmul operand dtype: F32R (accurate) or BF16 (fast)
AF = mybir.ActivationFunctionType
SPLIT_EXP = False
N_CORES = 8


def build_nc(loop_n=None, mm_dtype=None, phase='full', proj_bufs=2, copy_eng='dve'):
    MMD = mm_dtype or MM_DTYPE
    nc = bacc.Bacc("TRN2", target_bir_lowering=False, debug=False,
                   num_devices=N_CORES)
    xT = nc.dram_tensor("xT", [C, T], MMD, kind="ExternalInput")
    wq = nc.dram_tensor("wq", [C, CS], MMD, kind="ExternalInput")
    wk = nc.dram_tensor("wk", [C, CS], MMD, kind="ExternalInput")
    wv = nc.dram_tensor("wv", [C, CS], MMD, kind="ExternalInput")
    wp = nc.dram_tensor("wp", [CS, C], MMD, kind="ExternalInput")
    bq = nc.dram_tensor("bq", [P, 4], F32, kind="ExternalInput")
    mask = nc.dram_tensor("mask", [P, P], F32, kind="ExternalInput")
    y = nc.dram_tensor("y", [T, C], F16, kind="ExternalOutput")
    dbg = (nc.dram_tensor("dbg", [P, 3, 4224], MMD, kind="ExternalOutput")
           if phase != 'full' else None)

    with tile.TileContext(nc) as tc:
        with (
            tc.tile_pool(name="big", bufs=1) as big,
            tc.tile_pool(name="es_pool", bufs=3) as es_pool,
            tc.tile_pool(name="y_pool", bufs=3) as y_pool,
            tc.tile_pool(name="small", bufs=2) as small,
            tc.tile_pool(name="proj_ps", bufs=proj_bufs, space="PSUM") as proj_ps,
            tc.tile_pool(name="sc_ps", bufs=2, space="PSUM") as sc_ps,
            tc.tile_pool(name="ctx_ps", bufs=1, space="PSUM") as ctx_ps,
        ):
            from contextlib import ExitStack
            _ls = ExitStack()
            if loop_n:
                _ls.enter_context(tc.For_i(0, loop_n, 1))
            xT_sb = big.tile([P, 8, T], MMD)
            wq_sb = big.tile([P, 8, CS], MMD)
            wk_sb = big.tile([P, 8, CS], MMD)
            wv_sb = big.tile([P, 8, CS], MMD)
            wp_sb = big.tile([P, 4, C], MMD)
            bq_sb = big.tile([P, 4], F32)
            mask_sb = big.tile([P, P], F32)
            qT_sb = big.tile([P, 4, 2, 512], MMD)
            kT_sb = big.tile([P, 4, 2, 512], MMD)
            # V natural layout + 64-wide ones block per head: the ctx matmul
            # then emits the softmax denominator pre-broadcast across 64
            # partitions (stationary width is free on the PE). Ones block
            # first so the denominator lands at PSUM partition offset 0 —
            # custom DVE ops (reciprocal_approx_fast) require offset-0 APs
            # on hardware.
            v_sb = big.tile([P, 8, 8, 2, 64], MMD)
            ctxT_sb = big.tile([P, 4, T], MMD)

            nc.sync.dma_start(out=bq_sb, in_=bq.ap())
            nc.sync.dma_start(out=mask_sb, in_=mask.ap())
            nc.gpsimd.memset(v_sb[:, :, :, 0, :], 1.0)
            # Spread input loads across engine DMA queues: the rings execute
            # in parallel on HW, so xT/wv (gating the first V-proj matmuls)
            # stream on their own queues while wq/wk/wp fill others.
            xT_r = xT.ap().rearrange("(c p) t -> p c t", p=P)
            wq_r = wq.ap().rearrange("(c p) n -> p c n", p=P)
            wk_r = wk.ap().rearrange("(c p) n -> p c n", p=P)
            wv_r = wv.ap().rearrange("(c p) n -> p c n", p=P)
            for c in range(8):
                nc.sync.dma_start(out=xT_sb[:, c, :], in_=xT_r[:, c, :])
                nc.scalar.dma_start(out=wv_sb[:, c, :], in_=wv_r[:, c, :])
                nc.sync.dma_start(out=wq_sb[:, c, :], in_=wq_r[:, c, :])
                nc.scalar.dma_start(out=wk_sb[:, c, :], in_=wk_r[:, c, :])
            wp_r = wp.ap().rearrange("(k p) n -> p k n", p=P)
            for kc in range(4):
                nc.gpsimd.dma_start(out=wp_sb[:, kc, :], in_=wp_r[:, kc, :])

            # ---- V natural [T, 512] per head ----
            for tt in range(8):
                ps = proj_ps.tile([P, 512], F32, tag="proj")
                for c in range(8):
                    nc.tensor.matmul(
                        ps, xT_sb[:, c, tt * P:(tt + 1) * P], wv_sb[:, c, :],
                        start=(c == 0), stop=(c == 7))
                # Act engine is idle during the V-projection phase
                nc.scalar.copy(
                    v_sb[:, tt, :, 1, :],
                    ps.rearrange("p (h d) -> p h d", h=8))

            def qk_group(mc, wsb, outsb, is_q, tc2):
                ps = proj_ps.tile([P, 512], F32, tag="proj", name="qkps")
                for c in range(8):
                    nc.tensor.matmul(
                        ps, wsb[:, c, mc * P:(mc + 1) * P],
                        xT_sb[:, c, tc2 * 512:(tc2 + 1) * 512],
                        start=(c == 0), stop=(c == 7))
                dst = outsb[:, mc, tc2, :]
                if is_q:
                    nc.vector.tensor_add(
                        dst, ps, bq_sb[:, mc:mc + 1].broadcast_to([P, 512]))
                elif copy_eng == 'dve':
                    nc.vector.tensor_copy(dst, ps)
                else:
                    nc.scalar.copy(dst, ps)

            def qk_groups(mc):
                """One closure per PSUM group of qk_proj(mc): usable as PE
                filler inside a preceding attention call."""
                return [
                    (lambda mc=mc, w=wsb, o=outsb, q=is_q, t=tc2:
                     qk_group(mc, w, o, q, t))
                    for wsb, outsb, is_q in ((wq_sb, qT_sb, True),
                                             (wk_sb, kT_sb, False))
                    for tc2 in range(2)
                ]

            def qk_proj(mc):
                for g in qk_groups(mc):
                    g()

            def norm_write(h, qc, cps):
                # cps [128, 512]: rows 0-63 denominator (broadcast by the
                # ones block of V), rows 64-127 ctx.
                hp = (h % 2) * 64
                mc = h // 2
                recb = small.tile([64, 512], F32, tag="recb", name="recb")
                nc.vector.reciprocal_approx_fast(recb, cps[0:64, :])
                nc.vector.tensor_mul(
                    ctxT_sb[hp:hp + 64, mc, qc * 512:(qc + 1) * 512],
                    cps[64:128, :], recb)

            def attention(h, fillers=(), fill_every=3):
                # Software-pipelined: scores(kt+1) is emitted BEFORE ctx(kt)
                # so the in-order PE fills the exp/mask latency; `fillers`
                # (independent dense matmul groups, e.g. next mc's QK
                # projection) are drained between steps to keep the PE hot.
                hp = (h % 2) * 64
                mc = h // 2
                fil = list(fillers)
                cps0 = ctx_ps.tile([P, 512], F32, tag="ctx0", name="cps0")
                cps1 = ctx_ps.tile([P, 512], F32, tag="ctx1", name="cps1")
                pend = None          # deferred ctx emission for kt-1

                for kt in range(8):
                    lhsT = kT_sb[hp:hp + 64, mc, kt // 4,
                                 (kt % 4) * P:(kt % 4 + 1) * P]
                    sps = sc_ps.tile([P, 2, 512], F32, tag="sc", name="sps")
                    es = es_pool.tile([P, 2, 512], MMD, tag="es", name="es")
                    if kt < 4:
                        r0 = kt * P
                        nc.tensor.matmul(sps[:, 0, r0:], lhsT,
                                         qT_sb[hp:hp + 64, mc, 0, r0:],
                                         start=True, stop=True)
                        nc.tensor.matmul(sps[:, 1, :], lhsT,
                                         qT_sb[hp:hp + 64, mc, 1, :],
                                         start=True, stop=True)
                        if SPLIT_EXP:
                            nc.scalar.activation(es[:, 0, r0:], sps[:, 0, r0:],
                                                 AF.Exp)
                            nc.scalar.activation(es[:, 1, :], sps[:, 1, :],
                                                 AF.Exp)
                        else:
                            nc.scalar.activation(
                                es.rearrange("p a b -> p (a b)")[:, r0:],
                                sps.rearrange("p a b -> p (a b)")[:, r0:], AF.Exp)
                        nc.vector.tensor_mul(es[:, 0, r0:r0 + P],
                                             es[:, 0, r0:r0 + P], mask_sb)

                        def ctx_emit(kt=kt, es=es, r0=r0):
                            vl = v_sb[:, kt, h].rearrange("p a b -> p (a b)")
                            nc.tensor.matmul(cps0[:, r0:], vl, es[:, 0, r0:],
                                             start=(kt == 0), stop=(kt == 3))
                            nc.tensor.matmul(cps1, vl, es[:, 1, :],
                                             start=(kt == 0), stop=(kt == 7))
                            if kt == 3:
                                norm_write(h, 0, cps0)
                    else:
                        r0 = (kt - 4) * P
                        nc.tensor.matmul(sps[:, 0, r0:], lhsT,
                                         qT_sb[hp:hp + 64, mc, 1, r0:],
                                         start=True, stop=True)
                        nc.scalar.activation(es[:, 0, r0:], sps[:, 0, r0:], AF.Exp)
                        nc.vector.tensor_mul(es[:, 0, r0:r0 + P],
                                             es[:, 0, r0:r0 + P], mask_sb)

                        def ctx_emit(kt=kt, es=es, r0=r0):
                            vl = v_sb[:, kt, h].rearrange("p a b -> p (a b)")
                            nc.tensor.matmul(cps1[:, r0:], vl, es[:, 0, r0:],
                                             start=False, stop=(kt == 7))

                    if fil and kt % fill_every == (fill_every - 1):
                        fil.pop(0)()
                    if pend is not None:
                        pend()
                    pend = ctx_emit
                pend()
                for g in fil:
                    g()
                norm_write(h, 1, cps1)

            def yproj(tt_range):
                for tt in tt_range:
                    for nk in range(2):
                        ps = proj_ps.tile([P, 512], F32, tag="proj", name="yps")
                        for kc in range(4):
                            nc.tensor.matmul(
                                ps, ctxT_sb[:, kc, tt * P:(tt + 1) * P],
                                wp_sb[:, kc, nk * 512:(nk + 1) * 512],
                                start=(kc == 0), stop=(kc == 3))
                        ysb = y_pool.tile([P, 512], F16, tag="y", name="ysb")
                        nc.scalar.copy(ysb, ps)
                        nc.sync.dma_start(
                            out=y.ap()[tt * P:(tt + 1) * P,
                                       nk * 512:(nk + 1) * 512],
                            in_=ysb)

            if phase == 'dma':
                for di, sb_t in enumerate((xT_sb, wq_sb, wk_sb, wv_sb, wp_sb)):
                    nch = sb_t.shape[1]
                    nc.sync.dma_start(
                        out=dbg.ap()[:, 0, di * 64:di * 64 + nch * 8],
                        in_=sb_t[:, :, :8])
            elif phase == 'proj':
                for mc in range(4):
                    qk_proj(mc)
                nc.sync.dma_start(out=dbg.ap()[:, 0, :4096],
                                  in_=qT_sb.rearrange("p a b c -> p (a b c)"))
                nc.sync.dma_start(out=dbg.ap()[:, 1, :4096],
                                  in_=kT_sb.rearrange("p a b c -> p (a b c)"))
                nc.sync.dma_start(
                    out=dbg.ap()[:, 2, :4096],
                    in_=v_sb[:, :, :, 1, :].rearrange("p a b c -> p (a b c)"))
            elif phase == 'attn':
                for mc in range(4):
                    qk_proj(mc)
                    attention(2 * mc)
                    attention(2 * mc + 1)
                nc.sync.dma_start(out=dbg.ap()[:, 0, :4096],
                                  in_=ctxT_sb.rearrange("p a b -> p (a b)"))
            else:
                qk_proj(0)
                for mc in range(4):
                    nxt = qk_groups(mc + 1) if mc < 3 else []
                    attention(2 * mc, fillers=nxt[:2])
                    attention(2 * mc + 1, fillers=nxt[2:])
                    if mc == 3:
                        yproj(range(8))
            _ls.close()
    nc.compile()
    return nc


_NC = None


def _get_nc():
    global _NC
    if _NC is None:
        _NC = build_nc()
    return _NC


def make_in_maps(x, Wq, bq, Wk, Wv, Wp, mm_dtype=None):
    """Per-core input dicts."""
    import ml_dtypes
    MMD = mm_dtype or MM_DTYPE
    cvt = ((lambda a: np.ascontiguousarray(a).astype(ml_dtypes.bfloat16))
           if MMD == BF16 else np.ascontiguousarray)
    masks = (np.arange(P)[None, :] >= np.arange(P)[:, None]).astype(np.float32)
    in_maps = []
    for core in range(N_CORES):
        b = core // 2
        g = core % 2
        cs = slice(g * CS, (g + 1) * CS)
        in_maps.append(dict(
            xT=cvt(x[b].T),
            wq=cvt(Wq[:, cs] * np.float32(0.125)),
            wk=cvt(Wk[:, cs]),
            wv=cvt(Wv[:, cs]),
            wp=cvt(Wp[cs, :]),
            bq=np.ascontiguousarray((bq[cs] * np.float32(0.125))
                                    .reshape(4, P).T),
            mask=masks,
        ))
    return in_maps


def combine(parts, Wq, bv, Wp, bp):
    """parts: list of 8 per-core partial y arrays -> full [B, T, C] output."""
    out = np.stack([parts[2 * b].astype(np.float32)
                    + parts[2 * b + 1].astype(np.float32) for b in range(B)])
    out += (bv @ Wp + bp)[None, None, :]
    return out.astype(np.float32)


def kernel(**inputs):
    x = np.asarray(inputs["x"], np.float32)
    Wq = np.asarray(inputs["Wq"], np.float32)
    bq = np.asarray(inputs["bq"], np.float32)
    Wk = np.asarray(inputs["Wk"], np.float32)
    Wv = np.asarray(inputs["Wv"], np.float32)
    Wp = np.asarray(inputs["Wp"], np.float32)
    bv = np.asarray(inputs["bv"], np.float32)
    bp = np.asarray(inputs["bp"], np.float32)
    # bk intentionally unused: it shifts every score of a query row by the
    # same amount, which softmax cancels exactly.

    nc = _get_nc()
    in_maps = make_in_maps(x, Wq, bq, Wk, Wv, Wp)
    res = run_bass_kernel_spmd(nc, in_maps, core_ids=list(range(N_CORES)))
    parts = [res.results[c]["y"] for c in range(N_CORES)]
    return combine(parts, Wq, bv, Wp, bp)



# revision 42
# speedup vs baseline: 1.1427x; 1.1427x over previous
"""Multi-head causal attention (GPT-2 style) on 8 TRN2 NeuronCores.

Sharding: core i handles batch i//2 and head-group i%2 (8 of 16 heads,
i.e. a 512-wide slice of the QKV projections and of the Wp rows).  Each
core computes a partial output-projection for its batch; partials from
the two cores of a batch are summed on the host (cheap 4MB adds), along
with the exactly-factored bias terms:
  - bq is added to Q on-device (affects scores per key-column),
  - bk is dropped (adds a per-query constant to scores: softmax-invariant),
  - bv and bp commute through attention (rows of attn sum to 1):
    y += bv @ Wp + bp, applied on host.

On-chip layout (per core), T=1024, C=1024, DH=64:
  xT   [C, T]   x transposed (host-side transpose)         -> rhs / lhsT
  Q^T  [512, T] = (Wq_s*s)^T x^T  (s=1/8 folded into Wq)   -> scores rhs
  K^T  [512, T]                                            -> scores lhsT
  V    [T, 8, {ones64, data64}]: 64-wide ones block + V natural rows.
       The ctx matmul (stationary [128, 128]) then emits the softmax
       denominator PRE-BROADCAST across PSUM partitions 0-63 (stationary
       width is free on the PE), ctx data in partitions 64-127.  Ones
       first because custom DVE ops (reciprocal_approx_fast) require
       partition-offset-0 APs on hardware.
  S^T  [k-tile 128, q-chunk 512] scores transposed; causal mask applied
       as an elementwise multiply on exp(S^T) diagonal blocks.
  ctx^T[512, T] normalized context                         -> yproj lhsT
Matmuls in bf16, fp32 PSUM accumulation.  Attention is software-
pipelined: scores(kt+1) is emitted before ctx(kt) so the in-order PE
fills the exp/mask latency; the next mc's QK projection groups are
drained between attention steps as PE filler.  With unroll=2 (timing
loop), two bodies with ping-ponged input tiles run per hardware-loop
iteration so body N+1's input DMA streams during body N's compute.
"""
import numpy as np

import concourse.bacc as bacc
import concourse.mybir as mybir
import concourse.tile as tile
from concourse.bass_utils import run_bass_kernel_spmd

B, T, C, H, DH = 4, 1024, 1024, 16, 64
P = 128
CS = 512            # per-core head-slice width (8 heads * 64)
F32 = mybir.dt.float32
F16 = mybir.dt.float16
BF16 = mybir.dt.bfloat16
MM_DTYPE = BF16
AF = mybir.ActivationFunctionType
SPLIT_EXP = False
N_CORES = 8


def build_nc(loop_n=None, mm_dtype=None, phase='full', proj_bufs=2,
             copy_eng='dve', unroll=1):
    MMD = mm_dtype or MM_DTYPE
    nc = bacc.Bacc("TRN2", target_bir_lowering=False, debug=False,
                   num_devices=N_CORES)
    xT = nc.dram_tensor("xT", [C, T], MMD, kind="ExternalInput")
    wq = nc.dram_tensor("wq", [C, CS], MMD, kind="ExternalInput")
    wk = nc.dram_tensor("wk", [C, CS], MMD, kind="ExternalInput")
    wv = nc.dram_tensor("wv", [C, CS], MMD, kind="ExternalInput")
    wp = nc.dram_tensor("wp", [CS, C], MMD, kind="ExternalInput")
    bq = nc.dram_tensor("bq", [P, 4], F32, kind="ExternalInput")
    mask = nc.dram_tensor("mask", [P, P], MMD, kind="ExternalInput")
    y = nc.dram_tensor("y", [T, C], F16, kind="ExternalOutput")
    dbg = (nc.dram_tensor("dbg", [P, 3, 4224], MMD, kind="ExternalOutput")
           if phase != 'full' else None)

    with tile.TileContext(nc) as tc:
        with (
            tc.tile_pool(name="big", bufs=1) as big,
            tc.tile_pool(name="es_pool", bufs=3) as es_pool,
            tc.tile_pool(name="y_pool", bufs=3) as y_pool,
            tc.tile_pool(name="small", bufs=3) as small,
            tc.tile_pool(name="proj_ps", bufs=proj_bufs, space="PSUM") as proj_ps,
            tc.tile_pool(name="sc_ps", bufs=2, space="PSUM") as sc_ps,
            tc.tile_pool(name="ctx_ps", bufs=1, space="PSUM") as ctx_ps,
        ):
            from contextlib import ExitStack
            _ls = ExitStack()
            # shared across unrolled bodies (WAR-serialized by tile sems)
            bq_sb = big.tile([P, 4], F32)
            mask_sb = big.tile([P, P], MMD)
            qT_sb = big.tile([P, 4, 2, 512], MMD)
            kT_sb = big.tile([P, 4, 2, 512], MMD)
            v_sb = big.tile([P, 8, 8, 2, 64], MMD)
            ctxT_sb = big.tile([P, 4, T], MMD)
            # per-body ping-pong input tiles
            sets = [
                dict(
                    xT_sb=big.tile([P, 8, T], MMD, name=f"xT_sb{u}"),
                    wq_sb=big.tile([P, 8, CS], MMD, name=f"wq_sb{u}"),
                    wk_sb=big.tile([P, 8, CS], MMD, name=f"wk_sb{u}"),
                    wv_sb=big.tile([P, 8, CS], MMD, name=f"wv_sb{u}"),
                    wp_sb=big.tile([P, 4, C], MMD, name=f"wp_sb{u}"),
                )
                for u in range(unroll)
            ]
            if loop_n:
                _ls.enter_context(tc.For_i(0, loop_n, 1))

            def emit_loads(s, first):
                nc.sync.dma_start(out=bq_sb, in_=bq.ap())
                nc.sync.dma_start(out=mask_sb, in_=mask.ap())
                if first:
                    nc.gpsimd.memset(v_sb[:, :, :, 0, :], 1.0)
                xT_r = xT.ap().rearrange("(c p) t -> p c t", p=P)
                wq_r = wq.ap().rearrange("(c p) n -> p c n", p=P)
                wk_r = wk.ap().rearrange("(c p) n -> p c n", p=P)
                wv_r = wv.ap().rearrange("(c p) n -> p c n", p=P)
                for c in range(8):
                    nc.sync.dma_start(out=s['xT_sb'][:, c, :], in_=xT_r[:, c, :])
                    nc.sync.dma_start(out=s['wv_sb'][:, c, :], in_=wv_r[:, c, :])
                    nc.sync.dma_start(out=s['wq_sb'][:, c, :], in_=wq_r[:, c, :])
                    nc.sync.dma_start(out=s['wk_sb'][:, c, :], in_=wk_r[:, c, :])
                wp_r = wp.ap().rearrange("(k p) n -> p k n", p=P)
                for kc in range(4):
                    nc.sync.dma_start(out=s['wp_sb'][:, kc, :], in_=wp_r[:, kc, :])

            def vproj(s):
                for tt in range(8):
                    ps = proj_ps.tile([P, 512], F32, tag="proj")
                    for c in range(8):
                        nc.tensor.matmul(
                            ps, s['xT_sb'][:, c, tt * P:(tt + 1) * P],
                            s['wv_sb'][:, c, :],
                            start=(c == 0), stop=(c == 7))
                    if copy_eng == 'dve':
                        nc.vector.tensor_copy(
                            v_sb[:, tt, :, 1, :],
                            ps.rearrange("p (h d) -> p h d", h=8))
                    else:
                        nc.scalar.copy(
                            v_sb[:, tt, :, 1, :],
                            ps.rearrange("p (h d) -> p h d", h=8))

            def qk_group(s, mc, wsb_k, outsb, is_q, tc2):
                ps = proj_ps.tile([P, 512], F32, tag="proj", name="qkps")
                wsb = s[wsb_k]
                for c in range(8):
                    nc.tensor.matmul(
                        ps, wsb[:, c, mc * P:(mc + 1) * P],
                        s['xT_sb'][:, c, tc2 * 512:(tc2 + 1) * 512],
                        start=(c == 0), stop=(c == 7))
                dst = outsb[:, mc, tc2, :]
                if is_q:
                    nc.vector.tensor_add(
                        dst, ps, bq_sb[:, mc:mc + 1].broadcast_to([P, 512]))
                elif copy_eng == 'dve':
                    nc.vector.tensor_copy(dst, ps)
                else:
                    nc.scalar.copy(dst, ps)

            def qk_groups(s, mc):
                return [
                    (lambda mc=mc, w=wsb_k, o=outsb, q=is_q, t=tc2:
                     qk_group(s, mc, w, o, q, t))
                    for wsb_k, outsb, is_q in (('wq_sb', qT_sb, True),
                                               ('wk_sb', kT_sb, False))
                    for tc2 in range(2)
                ]

            def qk_proj(s, mc):
                for g in qk_groups(s, mc):
                    g()

            def norm_write(h, qc, cps):
                # cps [128, 512]: rows 0-63 denominator (broadcast by the
                # ones block of V), rows 64-127 ctx.
                hp = (h % 2) * 64
                mc = h // 2
                recb = small.tile([64, 512], F32, tag="recb", name="recb")
                nc.vector.reciprocal_approx_fast(recb, cps[0:64, :])
                nc.vector.tensor_mul(
                    ctxT_sb[hp:hp + 64, mc, qc * 512:(qc + 1) * 512],
                    cps[64:128, :], recb)

            def attention(h, fillers=(), fill_every=3):
                # Software-pipelined: scores(kt+1) before ctx(kt); fillers
                # (independent dense matmul groups) drain between steps.
                hp = (h % 2) * 64
                mc = h // 2
                fil = list(fillers)
                cps0 = ctx_ps.tile([P, 512], F32, tag="ctx0", name="cps0")
                cps1 = ctx_ps.tile([P, 512], F32, tag="ctx1", name="cps1")
                pend = None

                for kt in range(8):
                    lhsT = kT_sb[hp:hp + 64, mc, kt // 4,
                                 (kt % 4) * P:(kt % 4 + 1) * P]
                    sps = sc_ps.tile([P, 2, 512], F32, tag="sc", name="sps")
                    es = es_pool.tile([P, 2, 512], MMD, tag="es", name="es")
                    if kt < 4:
                        r0 = kt * P
                        nc.tensor.matmul(sps[:, 0, r0:], lhsT,
                                         qT_sb[hp:hp + 64, mc, 0, r0:],
                                         start=True, stop=True)
                        nc.tensor.matmul(sps[:, 1, :], lhsT,
                                         qT_sb[hp:hp + 64, mc, 1, :],
                                         start=True, stop=True)
                        if SPLIT_EXP:
                            nc.scalar.activation(es[:, 0, r0:], sps[:, 0, r0:],
                                                 AF.Exp)
                            nc.scalar.activation(es[:, 1, :], sps[:, 1, :],
                                                 AF.Exp)
                        else:
                            nc.scalar.activation(
                                es.rearrange("p a b -> p (a b)")[:, r0:],
                                sps.rearrange("p a b -> p (a b)")[:, r0:],
                                AF.Exp)
                        nc.vector.tensor_mul(es[:, 0, r0:r0 + P],
                                             es[:, 0, r0:r0 + P], mask_sb)

                        def ctx_emit(kt=kt, es=es, r0=r0):
                            vl = v_sb[:, kt, h].rearrange("p a b -> p (a b)")
                            nc.tensor.matmul(cps0[:, r0:], vl, es[:, 0, r0:],
                                             start=(kt == 0), stop=(kt == 3))
                            nc.tensor.matmul(cps1, vl, es[:, 1, :],
                                             start=(kt == 0), stop=(kt == 7))
                            if kt == 3:
                                norm_write(h, 0, cps0)
                    else:
                        r0 = (kt - 4) * P
                        nc.tensor.matmul(sps[:, 0, r0:], lhsT,
                                         qT_sb[hp:hp + 64, mc, 1, r0:],
                                         start=True, stop=True)
                        nc.scalar.activation(es[:, 0, r0:], sps[:, 0, r0:],
                                             AF.Exp)
                        nc.vector.tensor_mul(es[:, 0, r0:r0 + P],
                                             es[:, 0, r0:r0 + P], mask_sb)

                        def ctx_emit(kt=kt, es=es, r0=r0):
                            vl = v_sb[:, kt, h].rearrange("p a b -> p (a b)")
                            nc.tensor.matmul(cps1[:, r0:], vl, es[:, 0, r0:],
                                             start=False, stop=(kt == 7))

                    if fil and kt % fill_every == (fill_every - 1):
                        fil.pop(0)()
                    if pend is not None:
                        pend()
                    pend = ctx_emit
                pend()
                for g in fil:
                    g()
                norm_write(h, 1, cps1)

            def yproj(s, tt_range):
                for tt in tt_range:
                    for nk in range(2):
                        ps = proj_ps.tile([P, 512], F32, tag="proj", name="yps")
                        for kc in range(4):
                            nc.tensor.matmul(
                                ps, ctxT_sb[:, kc, tt * P:(tt + 1) * P],
                                s['wp_sb'][:, kc, nk * 512:(nk + 1) * 512],
                                start=(kc == 0), stop=(kc == 3))
                        ysb = y_pool.tile([P, 512], F16, tag="y", name="ysb")
                        if copy_eng == 'dve':
                            nc.vector.tensor_copy(ysb, ps)
                        else:
                            nc.scalar.copy(ysb, ps)
                        nc.sync.dma_start(
                            out=y.ap()[tt * P:(tt + 1) * P,
                                       nk * 512:(nk + 1) * 512],
                            in_=ysb)

            def body(s, first):
                emit_loads(s, first)
                vproj(s)
                qk_proj(s, 0)
                for mc in range(4):
                    nxt = qk_groups(s, mc + 1) if mc < 3 else []
                    attention(2 * mc, fillers=nxt[:2])
                    attention(2 * mc + 1, fillers=nxt[2:])
                    if mc == 3:
                        yproj(s, range(8))

            if phase == 'dma':
                emit_loads(sets[0], True)
                for di, k in enumerate(('xT_sb', 'wq_sb', 'wk_sb', 'wv_sb',
                                        'wp_sb')):
                    sb_t = sets[0][k]
                    nch = sb_t.shape[1]
                    nc.sync.dma_start(
                        out=dbg.ap()[:, 0, di * 64:di * 64 + nch * 8],
                        in_=sb_t[:, :, :8])
            elif phase == 'proj':
                emit_loads(sets[0], True)
                vproj(sets[0])
                for mc in range(4):
                    qk_proj(sets[0], mc)
                nc.sync.dma_start(out=dbg.ap()[:, 0, :4096],
                                  in_=qT_sb.rearrange("p a b c -> p (a b c)"))
                nc.sync.dma_start(out=dbg.ap()[:, 1, :4096],
                                  in_=kT_sb.rearrange("p a b c -> p (a b c)"))
                nc.sync.dma_start(out=dbg.ap()[:, 2, :64],
                                  in_=v_sb[:, 0, 0, 1, :])
            elif phase == 'attn':
                emit_loads(sets[0], True)
                vproj(sets[0])
                qk_proj(sets[0], 0)
                for mc in range(4):
                    nxt = qk_groups(sets[0], mc + 1) if mc < 3 else []
                    attention(2 * mc, fillers=nxt[:2])
                    attention(2 * mc + 1, fillers=nxt[2:])
                nc.sync.dma_start(out=dbg.ap()[:, 0, :4096],
                                  in_=ctxT_sb.rearrange("p a b -> p (a b)"))
            else:
                for u in range(unroll):
                    body(sets[u], first=(u == 0))
            _ls.close()
    nc.compile()
    return nc


_NC = None


def _get_nc():
    global _NC
    if _NC is None:
        _NC = build_nc()
    return _NC


def make_in_maps(x, Wq, bq, Wk, Wv, Wp, mm_dtype=None):
    """Per-core input dicts."""
    import ml_dtypes
    MMD = mm_dtype or MM_DTYPE
    cvt = ((lambda a: np.ascontiguousarray(a).astype(ml_dtypes.bfloat16))
           if MMD == BF16 else np.ascontiguousarray)
    masks = (np.arange(P)[None, :] >= np.arange(P)[:, None]).astype(np.float32)
    in_maps = []
    for core in range(N_CORES):
        b = core // 2
        g = core % 2
        cs = slice(g * CS, (g + 1) * CS)
        in_maps.append(dict(
            xT=cvt(x[b].T),
            wq=cvt(Wq[:, cs] * np.float32(0.125)),
            wk=cvt(Wk[:, cs]),
            wv=cvt(Wv[:, cs]),
            wp=cvt(Wp[cs, :]),
            bq=np.ascontiguousarray((bq[cs] * np.float32(0.125))
                                    .reshape(4, P).T),
            mask=cvt(masks),
        ))
    return in_maps


def combine(parts, Wq, bv, Wp, bp):
    """parts: list of 8 per-core partial y arrays -> full [B, T, C] output."""
    out = np.stack([parts[2 * b].astype(np.float32)
                    + parts[2 * b + 1].astype(np.float32) for b in range(B)])
    out += (bv @ Wp + bp)[None, None, :]
    return out.astype(np.float32)


def kernel(**inputs):
    x = np.asarray(inputs["x"], np.float32)
    Wq = np.asarray(inputs["Wq"], np.float32)
    bq = np.asarray(inputs["bq"], np.float32)
    Wk = np.asarray(inputs["Wk"], np.float32)
    Wv = np.asarray(inputs["Wv"], np.float32)
    Wp = np.asarray(inputs["Wp"], np.float32)
    bv = np.asarray(inputs["bv"], np.float32)
    bp = np.asarray(inputs["bp"], np.float32)
    # bk intentionally unused: it shifts every score of a query row by the
    # same amount, which softmax cancels exactly.

    nc = _get_nc()
    in_maps = make_in_maps(x, Wq, bq, Wk, Wv, Wp)
    res = run_bass_kernel_spmd(nc, in_maps, core_ids=list(range(N_CORES)))
    parts = [res.results[c]["y"] for c in range(N_CORES)]
    return combine(parts, Wq, bv, Wp, bp)
